# revision 18
# baseline (speedup 1.0000x reference)
"""Trainium2 Bass kernel for a 2-layer SLSTM net (T=1000, B=256, C=14, H=128, NC=7).

Computation (matches the reference nn.Module):
  spk1 = SLSTM(x; Wih1, Whh1, b1, thr1).spikes          [T, B, H]
  spk1n = BatchNorm_train(spk1; gamma, beta)            (stats over T*B)
  mem2 = SLSTM(spk1n; Wih2, Whh2, b2, thr2).mems        [T, B, H]
  out  = mem2.mean(0) @ Wfc.T + bfc                     [B, NC]

Sharding: data-parallel over batch across 8 NeuronCores (32 rows/core);
weights replicated; BN statistics all-reduced across cores (spikes are
binary so only the per-channel spike count is needed: var = mu - mu^2).

Per-core layout: H on partitions, batch on the free dim.  Per timestep:
  gates psum[:, (gate,b)] = Wg @ x_t(+bias via ones-row) + Wg_hh @ h
  sigma over (i,f,g) in one ACT instr (g pre-scaled x2 so tanh(g)=2*sig(2g)-1),
  fused DVE ops for the cell/hidden update, spikes stored as -thr*spk.
"""

import os
import sys
import numpy as np

T, B, C, H, NCLS = 1000, 256, 14, 128, 7
NCORES = 8
BL = B // NCORES  # 32 batch rows per core
BN_EPS = 1e-5

_PROGRAM_CACHE = {}


def _ensure_import():
    for p in ("/opt/trn_rl_repo",):
        if p not in sys.path and os.path.isdir(p):
            sys.path.insert(0, p)


def _build_program(t_steps, thr1, thr2):
    """Build the SPMD Bass program (same NEFF on all 8 cores)."""
    _ensure_import()
    import concourse.bass as bass  # noqa: F401
    from concourse import bacc
    import concourse.mybir as mybir
    from concourse.tile import TileContext

    f32 = mybir.dt.float32
    ALU = mybir.AluOpType
    AF = mybir.ActivationFunctionType
    AX = mybir.AxisListType

    nT = t_steps
    XPF = ((nT + 2) // 3) * BL  # free size of packed x (3 partition phases)

    nc = bacc.Bacc(
        "TRN2",
        target_bir_lowering=False,
        debug=False,
        num_devices=NCORES,
    )

    # ---- I/O ----
    # const blob columns: w1x(512) w1h(512) w2x(512) w2h(512) idt(128)
    #                     bias2(4) wfct(7) bfc(1) gamma(1) beta(1)
    CBLOB_F = 4 * (4 * H) + H + 4 + NCLS + 3
    xp = nc.declare_dram_parameter("xp", [79, XPF], f32, isOutput=False)
    cblob = nc.declare_dram_parameter("cblob", [H, CBLOB_F], f32, isOutput=False)
    out_d = nc.declare_dram_parameter("out", [NCLS, BL], f32, isOutput=True)

    # collective bounce buffers (internal DRAM)
    cc_in = nc.dram_tensor("cc_in", [H, 1], f32)
    cc_out = nc.dram_tensor("cc_out", [H, 1], f32, addr_space="Shared")

    TB = float(T * B)  # BN stat count (full batch, all cores)

    with TileContext(nc) as tc:
        with (
            tc.tile_pool(name="const", bufs=1) as cpool,
            tc.tile_pool(name="state", bufs=1) as spool,
            tc.tile_pool(name="work", bufs=2) as wpool,
            tc.tile_pool(name="ps_ifg", bufs=2, space="PSUM") as ps_ifg,
            tc.tile_pool(name="ps_o", bufs=2, space="PSUM") as ps_o,
            tc.tile_pool(name="ps_misc", bufs=1, space="PSUM") as ps_misc,
        ):
            # ---- load constants to SBUF (single blob DMA, sliced views) ----
            XP = cpool.tile([79, XPF], f32, tag="xp")
            CB_ALL = cpool.tile([H, CBLOB_F], f32, tag="cblob")
            off = [0]

            def blob(cols):
                a, off[0] = off[0], off[0] + cols
                return CB_ALL[:, a:a + cols]

            W1X_full = blob(4 * H)
            W1X = W1X_full  # [H, 512]; only partitions 0..78 meaningful
            W1H = blob(4 * H)
            W2X = blob(4 * H)
            W2H = blob(4 * H)
            IDT = blob(H)
            BIAS2 = blob(4)
            WFCT = blob(NCLS)
            BFC_col = blob(1)  # [H,1]; first NCLS partitions hold bfc
            GAM = blob(1)
            BET = blob(1)
            W2A = cpool.tile([H, 4 * H], f32, tag="w2a")
            CONST2 = cpool.tile([H, 4 * BL], f32, tag="const2")
            Z32 = cpool.tile([H, BL], f32, tag="z32")
            BFC = BFC_col

            # big persistent buffers
            SPK1 = spool.tile([H, nT * BL], f32, tag="spk1")  # -thr1 * spk1
            S = spool.tile([H, 2 * BL], f32, tag="s")  # [tg | c]
            SBG = spool.tile([H, 4 * BL], f32, tag="sbg")  # sigma outs
            T12 = spool.tile([H, 2 * BL], f32, tag="t12")
            TC = spool.tile([H, BL], f32, tag="tc")
            U = spool.tile([H, BL], f32, tag="u")
            HB = spool.tile([H, 2 * BL], f32, tag="hb")  # h double buffer
            RING2 = spool.tile([H, 2 * BL], f32, tag="ring2")  # scan2 spkneg
            MEMSUM = spool.tile([H, BL], f32, tag="memsum")
            CNT = spool.tile([H, 1], f32, tag="cnt")
            CNTG = spool.tile([H, 1], f32, tag="cntg")

            dma = nc.sync.dma_start
            dma(out=XP[:, :], in_=xp[:, :])
            dma(out=CB_ALL[:, :], in_=cblob[:, :])

            nc.vector.memset(Z32[:, :], 0.0)
            nc.vector.memset(S[:, :], 0.0)
            nc.vector.memset(HB[:, :], 0.0)
            nc.vector.memset(MEMSUM[:, :], 0.0)
            nc.vector.memset(RING2[:, :], 0.0)

            tc.strict_bb_all_engine_barrier()

            mm = nc.tensor.matmul
            act = nc.scalar.activation
            tt = nc.vector.tensor_tensor
            ts = nc.vector.tensor_scalar
            stt = nc.vector.scalar_tensor_tensor

            def scan_step(t, thr, wx_t, wh_t, spk_dst, spk_prev, layer):
                """One SLSTM step. Gate free-order in psum: i | f | g | o."""
                gi = ps_ifg.tile([H, 3 * BL], f32, tag="g_ifg")
                go = ps_o.tile([H, BL], f32, tag="g_o")
                h_prev = HB[:, (t % 2) * BL:(t % 2 + 1) * BL]
                h_cur_sl = ((t + 1) % 2) * BL
                h_cur = HB[:, h_cur_sl:h_cur_sl + BL]

                # ---- x-part / const-part matmuls (off critical path) ----
                if layer == 1:
                    r, j = t % 3, t // 3
                    xrhs = XP[32 * r:32 * r + 15, BL * j:BL * j + BL]
                    for k, g in enumerate((0, 1, 2)):
                        mm(gi[:, g * BL:(g + 1) * BL],
                           wx_t[32 * r:32 * r + 15, g * H:(g + 1) * H], xrhs,
                           start=(k == 0), stop=False)
                    mm(go[:, :], wx_t[32 * r:32 * r + 15, 3 * H:4 * H], xrhs,
                       start=True, stop=False)
                else:
                    # bias/BN const via identity matmul, then spk1 projection
                    mm(gi[:, :], IDT[:, :], CONST2[:, 0:3 * BL],
                       start=True, stop=False)
                    mm(go[:, :], IDT[:, :], CONST2[:, 3 * BL:4 * BL],
                       start=True, stop=False)
                    sp_rhs = SPK1[:, t * BL:(t + 1) * BL]
                    for g in (0, 1, 2):
                        mm(gi[:, g * BL:(g + 1) * BL],
                           wx_t[:, g * H:(g + 1) * H], sp_rhs,
                           start=False, stop=False)
                    mm(go[:, :], wx_t[:, 3 * H:4 * H], sp_rhs, start=False, stop=False)

                # ---- recurrent matmuls (on critical path): g first, then i, f ----
                for last, g in ((False, 2), (False, 0), (True, 1)):
                    mm(gi[:, g * BL:(g + 1) * BL],
                       wh_t[:, g * H:(g + 1) * H], h_prev,
                       start=False, stop=last)
                mm(go[:, :], wh_t[:, 3 * H:4 * H], h_prev, start=False, stop=True)

                # ---- activations ----
                act(SBG[:, 0:3 * BL], gi[:, :], AF.Sigmoid)     # sig(i), sig(f), sig(2g)
                act(SBG[:, 3 * BL:4 * BL], go[:, :], AF.Sigmoid)  # sig(o)

                # tg = 2*sig(2g) - 1  -> S[:, 0:BL]
                ts(S[:, 0:BL], SBG[:, 2 * BL:3 * BL], 2.0, -1.0, ALU.mult, ALU.add)
                # [sig(i)*tg | sig(f)*c]
                tt(T12[:, :], SBG[:, 0:2 * BL], S[:, 0:2 * BL], ALU.mult)
                # c' = t1 + t2 (in place into S[:, BL:2BL])
                tt(S[:, BL:2 * BL], T12[:, 0:BL], T12[:, BL:2 * BL], ALU.add)
                act(TC[:, :], S[:, BL:2 * BL], AF.Tanh)
                # u = sig(o) * tanh(c)
                stt(U[:, :], SBG[:, 3 * BL:4 * BL], 1.0, TC[:, :], ALU.mult, ALU.mult)
                # h = u + (-thr * spk_prev)
                tt(h_cur, U[:, :], spk_prev, ALU.add)
                # spkneg = (h > thr) * (-thr)
                ts(spk_dst, h_cur, float(thr), -float(thr), ALU.is_gt, ALU.mult)
                if layer == 2:
                    tt(MEMSUM[:, :], MEMSUM[:, :], h_cur, ALU.add)

            # ================= scan 1 =================
            for t in range(nT):
                spk_prev = Z32[:, :] if t == 0 else SPK1[:, (t - 1) * BL:t * BL]
                scan_step(t, thr1, W1X, W1H, SPK1[:, t * BL:(t + 1) * BL],
                          spk_prev, layer=1)

            # ================= batch norm stats =================
            nc.vector.tensor_reduce(CNT[:, :], SPK1[:, :], AX.X, ALU.add)
            dma(out=cc_in[:, :], in_=CNT[:, :])
            nc.gpsimd.collective_compute(
                "AllReduce", ALU.add,
                replica_groups=[list(range(NCORES))],
                ins=[cc_in.ap()], outs=[cc_out.ap()],
            )
            dma(out=CNTG[:, :], in_=cc_out[:, :])

            MU = spool.tile([H, 1], f32, tag="mu")
            VAR = spool.tile([H, 1], f32, tag="var")
            D = spool.tile([H, 1], f32, tag="d")
            Y = spool.tile([H, 1], f32, tag="y")
            SC1 = spool.tile([H, 1], f32, tag="sc1")
            SC2 = spool.tile([H, 1], f32, tag="sc2")
            A = spool.tile([H, 1], f32, tag="a")
            AEFF = spool.tile([H, 1], f32, tag="aeff")
            BBN = spool.tile([H, 1], f32, tag="bbn")

            # mu = cnt * (-1/(thr1*T*B));  spk stored as -thr1*spk
            ts(MU[:, :], CNTG[:, :], float(-1.0 / (thr1 * TB)), None, ALU.mult)
            # var = mu * (1 - mu)
            ts(SC1[:, :], MU[:, :], -1.0, 1.0, ALU.mult, ALU.add)
            tt(VAR[:, :], SC1[:, :], MU[:, :], ALU.mult)
            ts(D[:, :], VAR[:, :], float(BN_EPS), None, ALU.add)
            # y = rsqrt(d): sqrt + reciprocal + 2 Newton iters
            act(SC1[:, :], D[:, :], AF.Sqrt)
            nc.vector.reciprocal(Y[:, :], SC1[:, :])
            for _ in range(2):
                tt(SC1[:, :], Y[:, :], Y[:, :], ALU.mult)
                tt(SC2[:, :], SC1[:, :], D[:, :], ALU.mult)
                ts(SC1[:, :], SC2[:, :], -0.5, 1.5, ALU.mult, ALU.add)
                tt(Y[:, :], Y[:, :], SC1[:, :], ALU.mult)
            # a = gamma * y ; aeff = a * (-1/thr1)
            tt(A[:, :], Y[:, :], GAM[:, :], ALU.mult)
            ts(AEFF[:, :], A[:, :], float(-1.0 / thr1), None, ALU.mult)
            # b_bn = beta - mu * a
            stt(SC1[:, :], MU[:, :], -1.0, A[:, :], ALU.mult, ALU.mult)
            tt(BBN[:, :], SC1[:, :], BET[:, :], ALU.add)

            # W2A = W2X scaled per input-channel (partition) by aeff
            act(W2A[:, :], W2X[:, :], AF.Copy, scale=AEFF[:, 0:1])

            # const2[h, (g,b)] = bias2[h, g] + (b_bn @ Wih2_g)[h], broadcast over b
            PCB = ps_misc.tile([H, 4], f32, tag="pcb")
            for g in range(4):
                mm(PCB[:, g:g + 1], W2X[:, g * H:(g + 1) * H], BBN[:, :],
                   start=(g == 0), stop=(g == 3))
            CB = spool.tile([H, 4], f32, tag="cb")
            tt(CB[:, :], PCB[:, :], BIAS2[:, :], ALU.add)
            for g in range(4):
                act(CONST2[:, g * BL:(g + 1) * BL], Z32[:, :], AF.Identity,
                    bias=CB[:, g:g + 1])

            # reset states for scan 2
            nc.vector.memset(S[:, :], 0.0)
            nc.vector.memset(HB[:, :], 0.0)

            tc.strict_bb_all_engine_barrier()

            # ================= scan 2 =================
            for t in range(nT):
                # slot (t-1)%2 == (t+1)%2; at t=0 RING2 is zeroed
                spk_prev = RING2[:, ((t + 1) % 2) * BL:((t + 1) % 2) * BL + BL]
                spk_dst = RING2[:, (t % 2) * BL:(t % 2) * BL + BL]
                scan_step(t, thr2, W2A, W2H, spk_dst, spk_prev, layer=2)

            # ================= final FC =================
            PF = ps_misc.tile([NCLS, BL], f32, tag="pf")
            mm(PF[:, :], WFCT[:, :], MEMSUM[:, :], start=True, stop=True)
            OUTS = spool.tile([NCLS, BL], f32, tag="outs")
            act(OUTS[:, :], PF[:, :], AF.Identity, bias=BFC[0:NCLS, 0:1])
            dma(out=out_d[:, :], in_=OUTS[:, :])

    nc.compile()
    return nc


def _pack_inputs(x, Wih1, Whh1, bih1, bhh1, Wih2, Whh2, bih2, bhh2,
                 gamma, beta, Wfc, bfc, t_steps):
    """Host-side packing into matmul-ready layouts (shared across cores
    except xp)."""
    nT = t_steps
    # gate g (index 2) pre-scaled by 2 for tanh(g) = 2*sigmoid(2g) - 1
    gsc = np.ones(4, np.float32)
    gsc[2] = 2.0

    def pack_w(Wih_like, K):
        # returns [K, 4H]: out[k, g*H + h] = W[g*H + h, k] * gsc[g]
        W = Wih_like.reshape(4, H, K)  # [g, h, k]
        W = W * gsc[:, None, None]
        return np.ascontiguousarray(W.transpose(2, 0, 1).reshape(K, 4 * H))

    b1 = (bih1 + bhh1).reshape(4, H) * gsc[:, None]  # [g, h]
    w1x = np.zeros((H, 4 * H), np.float32)
    for r in range(3):  # replicated at partition bases 0/32/64
        w1x[32 * r:32 * r + 14, :] = pack_w(Wih1, C)
        w1x[32 * r + 14, :] = b1.reshape(4 * H)
    w1h = pack_w(Whh1, H)
    w2x = pack_w(Wih2, H)
    w2h = pack_w(Whh2, H)
    bias2 = ((bih2 + bhh2).reshape(4, H) * gsc[:, None]).T  # [H, 4]
    wfct = np.zeros((H, NCLS), np.float32)
    wfct[:, :] = Wfc.T / float(t_steps)
    bfc_c = np.zeros((H, 1), np.float32)
    bfc_c[:NCLS, 0] = bfc
    gam = gamma.reshape(H, 1)
    bet = beta.reshape(H, 1)
    cblob = np.concatenate(
        [w1x, w1h, w2x, w2h, np.eye(H, dtype=np.float32),
         bias2, wfct, bfc_c, gam, bet], axis=1)

    # per-core packed x with ones row: xp[32*(t%3)+c, 32*(t//3)+b]
    xps = []
    nj = (nT + 2) // 3
    for core in range(NCORES):
        xc = x[:nT, core * BL:(core + 1) * BL, :]  # [nT, 32, 14]
        xp = np.zeros((79, nj * BL), np.float32)
        for r in range(3):
            xt = xc[r::3]  # steps r, r+3, ...
            njr = xt.shape[0]
            blk = np.ascontiguousarray(xt.transpose(2, 0, 1))  # [14, njr, 32]
            xp[32 * r:32 * r + 14, :njr * BL] = blk.reshape(14, njr * BL)
            xp[32 * r + 14, :njr * BL] = 1.0
        xps.append(xp)

    cblob = np.ascontiguousarray(cblob, dtype=np.float32)
    return [dict(cblob=cblob, xp=xps[c]) for c in range(NCORES)]


def kernel(x, Wih1, Whh1, bih1, bhh1, thr1, Wih2, Whh2, bih2, bhh2, thr2,
           gamma, beta, Wfc, bfc, _t_steps=T, _results_hook=None):
    _ensure_import()
    from concourse.bass_utils import run_bass_kernel_spmd

    args = [np.asarray(a, dtype=np.float32) for a in
            (x, Wih1, Whh1, bih1, bhh1, Wih2, Whh2, bih2, bhh2,
             gamma, beta, Wfc, bfc)]
    (x, Wih1, Whh1, bih1, bhh1, Wih2, Whh2, bih2, bhh2,
     gamma, beta, Wfc, bfc) = args
    thr1 = float(np.asarray(thr1)); thr2 = float(np.asarray(thr2))

    key = (_t_steps, thr1, thr2)
    if key not in _PROGRAM_CACHE:
        _PROGRAM_CACHE[key] = _build_program(_t_steps, thr1, thr2)
    nc = _PROGRAM_CACHE[key]

    in_maps = _pack_inputs(x, Wih1, Whh1, bih1, bhh1, Wih2, Whh2, bih2, bhh2,
                           gamma, beta, Wfc, bfc, _t_steps)
    res = run_bass_kernel_spmd(nc, in_maps, list(range(NCORES)))
    if _results_hook is not None:
        _results_hook(res)
    out = np.empty((B, NCLS), np.float32)
    for c in range(NCORES):
        out[c * BL:(c + 1) * BL, :] = res.results[c]["out"].T
    return out
